# revision 25
# baseline (speedup 1.0000x reference)
"""Trainium2 kernel for nn_BalancedHamiltonLayer — no-collective 2x4 sharding.

Math: out = einsum("btd,rde->bte", x, factors)/sqrt(rank) + bias.
The einsum contracts r as a plain sum, so sum_r (x @ F_r) == x @ (sum_r F_r):
one [16384,2048] @ [2048,2048] GEMM instead of eight.

Distribution over 8 NeuronCores (single SPMD program, no collectives):
  core c = (mh, eq) with mh = c//4, eq = c%4 owns
    x rows   [mh*8192, (mh+1)*8192)      (m-sharding x2)
    e-cols   [eq*512, (eq+1)*512)        (e-sharding x4)
  Each core loads its own factor slice [8, 2048, 512] and reduces
  W_c = sum_r F_r[:, eq-slice] on DVE — fully independent cores, no
  AllGather on the critical path.

Schedule notes (from perfetto traces; ~284-294us vs 389us for the
previous AllGather design):
  - Early aggregate DMA is ~300-350 GB/s per core (HBM pair-shared);
    per-engine descriptor rings are FIFO, so issue order is priority:
    fh0, x-q0 first half, fh1, x-q0 second half, fh2, fh3, x-q1..3.
    All input loads use >=16 KB per-partition lines.
  - Factors load per (e-tile, d-half) into separate W tiles so the GEMM
    starts on the first half-reduced W (~28us) while the rest lands.
  - GEMM groups 4 m-chunks of 512 per stationary W d-tile: LDWEIGHTS
    drops 4x (the 128-cycle weight swap per 512 rows cost 25% PE time
    in the per-matmul-LDWEIGHTS version); clean groups run 13.8us =
    ~99% of the bf16 PE roofline (218.5us/core total).
  - PSUM [128,4,512] double-buffered (8 banks). Per-bank evictions
    (scale+bias+bf16 in one pass) alternate Activation/Vector engines
    so they pipeline behind each bank's d=15 matmul; out-DMA in two
    halves on the gpsimd ring; the final group runs as two G=2
    sub-groups so its epilogue overlaps its own matmuls.
  Output bf16 out^T [512e, 8192m]; host transposes back to fp32.
"""

import math
import os

import numpy as np
import ml_dtypes

REPEAT = int(os.environ.get("BASS_BENCH_REPEAT", "1"))  # >1 only for benching
B, T, DIM, RANK = 4, 4096, 2048, 8
N_CORES = 8
MG, EG = 2, 4                  # m-groups x e-groups of cores
MROWS = (B * T) // MG          # 8192 rows per core
EC = DIM // EG                 # 512 e-cols per core
NQ = 4                         # x quarters per core
NMC = (MROWS // NQ) // 512     # 4 m-chunks of 512 per quarter
NT = DIM // 128                # 16 contraction tiles
NTH = NT // 2                  # 8 d-tiles per half
NET = EC // 128                # 4 e-tiles per core
SCALE = 1.0 / math.sqrt(RANK)

BF16 = ml_dtypes.bfloat16
_CACHE = {}


def _build():
    import concourse.bacc as bacc
    import concourse.mybir as mybir
    import concourse.tile as tile

    f32 = mybir.dt.float32
    bf16 = mybir.dt.bfloat16

    nc = bacc.Bacc(
        "TRN2", target_bir_lowering=False, debug=False, num_devices=N_CORES
    )
    # x^T quarters: [q, p, t, mc, m] with d = t*128+p, row = q*2048 + mc*512 + m
    xh = nc.dram_tensor(
        "xh", [NQ, 128, NT, NMC, 512], bf16, kind="ExternalInput"
    ).ap()
    # factor slice: [et, dh, p, r, th, e] with d = dh*1024 + th*128 + p,
    # e_local = et*128+e.  One 2 MB dma per (et, dh), 16 KB per-partition
    # lines.
    fh = nc.dram_tensor(
        "fh", [NET, 2, 128, RANK, NTH, 128], bf16, kind="ExternalInput"
    ).ap()
    # bias_cols[p, et] = bias[eq*512 + et*128 + p]
    bias_cols = nc.dram_tensor("bias_cols", [128, NET], f32, kind="ExternalInput").ap()
    # transposed output: outT[e_local, m_local], bf16
    outT = nc.dram_tensor("outT", [EC, MROWS], bf16, kind="ExternalOutput").ap()

    with tile.TileContext(nc) as tc:
        with (
            tc.tile_pool(name="const", bufs=1) as const_pool,
            tc.tile_pool(name="xa", bufs=2) as xa_pool,
            tc.tile_pool(name="fr", bufs=3) as fr_pool,
            tc.tile_pool(name="acc", bufs=1) as acc_pool,
            tc.tile_pool(name="w", bufs=8) as w_pool,
            tc.tile_pool(name="osb", bufs=2) as o_pool,
            tc.tile_pool(name="ps", bufs=2, space="PSUM") as p_pool,
        ):
            scope = nc.named_scope
            bias_sb = const_pool.tile([128, NET], f32)
            nc.sync.dma_start(bias_sb[:], bias_cols[:])

            for it in range(REPEAT):
                # wts[et*2 + dh] = W[d in half dh, e-tile et], [128, NTH, 128]
                wts = [None] * (NET * 2)
                fr_tiles = {}

                def issue_fh(et):
                    # both d-halves on the fh-dedicated rings (16 KB lines)
                    for dh in range(2):
                        fr = fr_pool.tile(
                            [128, RANK, NTH, 128], bf16, tag="fr",
                            name=f"fr{it}_{et}_{dh}",
                        )
                        eng = nc.sync if dh == 0 else nc.scalar
                        eng.dma_start(fr[:], fh[et, dh])
                        fr_tiles[(et, dh)] = fr

                def reduce_chains(et):
                    # serial DVE chain per d-half (~4.9us after the fh
                    # piece lands; GpSimd "parallel" halves measured
                    # slower — Pool adds have high per-op cost)
                    for dh in range(2):
                        fr = fr_tiles[(et, dh)]
                        with scope(f"red{it}_{et}_{dh}"):
                            a1 = acc_pool.tile(
                                [128, NTH, 128], bf16, tag="a1",
                                name=f"a1_{it}_{et}_{dh}",
                            )
                            nc.vector.tensor_add(a1[:], fr[:, 0], fr[:, 1])
                            for r in range(2, RANK - 1):
                                nc.vector.tensor_add(a1[:], a1[:], fr[:, r])
                            w = w_pool.tile(
                                [128, NTH, 128], bf16, tag="w",
                                name=f"w{it}_{et}_{dh}",
                            )
                            nc.vector.tensor_add(w[:], a1[:], fr[:, RANK - 1])
                            wts[et * 2 + dh] = w

                def issue_x(q, xa, pieces):
                    # pieces of 4 d-tiles (2 MB, 16 KB lines) alternating
                    # the two input rings; the d-loop consumes them in
                    # order. (Aggregate early DMA is ~300-330 GB/s per
                    # core — HBM pair-shared — so ring games don't help;
                    # this interleave empirically starts the PE earliest.)
                    for h in pieces:
                        eng = nc.sync if h % 2 == 0 else nc.scalar
                        eng.dma_start(
                            xa[:, 4 * h : 4 * h + 4], xh[q, :, 4 * h : 4 * h + 4]
                        )

                xa_tiles = []

                def new_xa(q):
                    t = xa_pool.tile(
                        [128, NT, NMC, 512], bf16, tag="xa", name=f"xa{it}_{q}"
                    )
                    xa_tiles.append(t)
                    return t

                # Ring order (per-ring FIFO): fh0, all of x0, fh1, fh2,
                # fh3, x1-3. The early phase is supply-bound either way;
                # loading x0 contiguously consolidates the unavoidable
                # supply stalls into ONE gap (before et1) instead of four,
                # so the PE p-state ramp (~1.5us per restart) is paid once.
                # Outs ride gpsimd.
                issue_fh(0)
                xa0 = new_xa(0)
                issue_x(0, xa0, [0, 1, 2, 3])
                issue_fh(1)
                issue_fh(2)
                issue_fh(3)
                for q in range(1, NQ):
                    issue_x(q, new_xa(q), [0, 1, 2, 3])
                reduce_chains(0)
                reduce_chains(1)

                # GEMM: out^T[e-tile, quarter]; 4 m-chunks share each
                # stationary W d-tile. et2/et3 reduce chains are issued
                # between the first GEMM groups so no engine queue holds
                # early evictions hostage behind late-gated chain adds.
                for q in range(NQ):
                    xa = xa_tiles[q]
                    for et in range(NET):
                        if q == 0 and et == 1:
                            reduce_chains(2)
                        if q == 0 and et == 2:
                            reduce_chains(3)
                        last = q == NQ - 1 and et == NET - 1
                        with scope(f"g{it}_{q}_{et}"):
                            ps = p_pool.tile([128, NMC, 512], f32, tag="ps")
                            if last:
                                # final group: two G=2 sub-groups so the
                                # first half's evictions + out-DMA overlap
                                # the second half's matmuls (shorter tail)
                                mc_waves = [[0, 1], [2, 3]]
                            else:
                                mc_waves = [list(range(NMC))]
                            for wave in mc_waves:
                                for d in range(NT):
                                    wt = wts[et * 2 + d // NTH]
                                    for mc in wave:
                                        nc.tensor.matmul(
                                            ps[:, mc, :],
                                            wt[:, d % NTH, :],
                                            xa[:, d, mc, :],
                                            start=(d == 0),
                                            stop=(d == NT - 1),
                                        )
                            osb = o_pool.tile([128, NMC * 512], bf16, tag="osb")
                            for mc in range(NMC):
                                # q0 evictions stay on Scalar: the Vector
                                # queue is still draining reduce chains and
                                # would hold the PSUM release hostage.
                                dst = osb[:, mc * 512 : (mc + 1) * 512]
                                src = ps[:, mc, :]
                                if q == 0 or mc % 2:
                                    nc.scalar.activation(
                                        dst,
                                        src,
                                        mybir.ActivationFunctionType.Identity,
                                        bias=bias_sb[:, et : et + 1],
                                        scale=SCALE,
                                    )
                                else:
                                    nc.vector.tensor_scalar(
                                        dst,
                                        src,
                                        SCALE,
                                        bias_sb[:, et : et + 1],
                                        mybir.AluOpType.mult,
                                        mybir.AluOpType.add,
                                    )
                            e0 = et * 128
                            m0 = q * NMC * 512
                            # two half-DMAs on the gpsimd ring (never behind
                            # WAR-gated x pieces): the last half only waits
                            # on the last two bank evictions, shortening the
                            # tail.
                            half = NMC * 512 // 2
                            nc.gpsimd.dma_start(
                                outT[e0 : e0 + 128, m0 : m0 + half], osb[:, :half]
                            )
                            nc.gpsimd.dma_start(
                                outT[e0 : e0 + 128, m0 + half : m0 + 2 * half],
                                osb[:, half:],
                            )

    nc.compile()
    return nc


def _get_nc():
    if "nc" not in _CACHE:
        _CACHE["nc"] = _build()
    return _CACHE["nc"]


def _shard(x, factors, bias):
    x_flat = np.ascontiguousarray(x, dtype=np.float32).reshape(B * T, DIM)
    factors = np.ascontiguousarray(factors, dtype=np.float32)
    bias = np.ascontiguousarray(bias, dtype=np.float32)
    in_maps = []
    for c in range(N_CORES):
        mh, eq = divmod(c, EG)
        xc = x_flat[mh * MROWS : (mh + 1) * MROWS, :]       # [m, d]
        # -> [q, p, t, mc, m512]
        xhc = np.ascontiguousarray(
            xc.reshape(NQ, NMC, 512, NT, 128).transpose(0, 4, 3, 1, 2).astype(BF16)
        )
        fc = factors[:, :, eq * EC : (eq + 1) * EC]          # [r, d, e]
        # -> [et, dh, p, r, th, e128]
        fhc = np.ascontiguousarray(
            fc.reshape(RANK, 2, NTH, 128, NET, 128)
            .transpose(4, 1, 3, 0, 2, 5)
            .astype(BF16)
        )
        bias_c = np.ascontiguousarray(
            bias[eq * EC : (eq + 1) * EC].reshape(NET, 128).T
        )
        in_maps.append({"xh": xhc, "fh": fhc, "bias_cols": bias_c})
    return in_maps


def _run(in_maps, trace=False, trace_cores=None):
    from concourse.bass_utils import run_bass_kernel_spmd

    nc = _get_nc()
    return run_bass_kernel_spmd(
        nc, in_maps, list(range(N_CORES)), trace=trace, trace_cores=trace_cores
    )


def _assemble(res):
    out = np.empty((B * T, DIM), dtype=np.float32)
    for c in range(N_CORES):
        mh, eq = divmod(c, EG)
        out[mh * MROWS : (mh + 1) * MROWS, eq * EC : (eq + 1) * EC] = (
            res.results[c]["outT"].T.astype(np.float32)
        )
    return out.reshape(B, T, DIM)


def kernel(x, factors, bias):
    res = _run(_shard(x, factors, bias), trace=False)
    return _assemble(res)


# revision 26
# speedup vs baseline: 1.0243x; 1.0243x over previous
"""Trainium2 kernel for nn_BalancedHamiltonLayer — no-collective 2x4 sharding.

Math: out = einsum("btd,rde->bte", x, factors)/sqrt(rank) + bias.
The einsum contracts r as a plain sum, so sum_r (x @ F_r) == x @ (sum_r F_r):
one [16384,2048] @ [2048,2048] GEMM instead of eight.

Distribution over 8 NeuronCores (single SPMD program, no collectives):
  core c = (mh, eq) with mh = c//4, eq = c%4 owns
    x rows   [mh*8192, (mh+1)*8192)      (m-sharding x2)
    e-cols   [eq*512, (eq+1)*512)        (e-sharding x4)
  Each core loads its own factor slice [8, 2048, 512] and reduces
  W_c = sum_r F_r[:, eq-slice] on DVE — fully independent cores, no
  AllGather on the critical path.

Schedule notes (from perfetto traces; ~284-294us vs 389us for the
previous AllGather design):
  - Early aggregate DMA is ~300-350 GB/s per core (HBM pair-shared);
    per-engine descriptor rings are FIFO, so issue order is priority:
    fh0, x-q0 first half, fh1, x-q0 second half, fh2, fh3, x-q1..3.
    All input loads use >=16 KB per-partition lines.
  - Factors load per (e-tile, d-half) into separate W tiles so the GEMM
    starts on the first half-reduced W (~28us) while the rest lands.
  - GEMM groups 4 m-chunks of 512 per stationary W d-tile: LDWEIGHTS
    drops 4x (the 128-cycle weight swap per 512 rows cost 25% PE time
    in the per-matmul-LDWEIGHTS version); clean groups run 13.8us =
    ~99% of the bf16 PE roofline (218.5us/core total).
  - PSUM [128,4,512] double-buffered (8 banks). Per-bank evictions
    (scale+bias+bf16 in one pass) alternate Activation/Vector engines
    so they pipeline behind each bank's d=15 matmul; out-DMA in two
    halves on the gpsimd ring; the final group runs as two G=2
    sub-groups so its epilogue overlaps its own matmuls.
  Output bf16 out^T [512e, 8192m]; host transposes back to fp32.
"""

import math
import os

import numpy as np
import ml_dtypes

REPEAT = int(os.environ.get("BASS_BENCH_REPEAT", "1"))  # >1 only for benching
B, T, DIM, RANK = 4, 4096, 2048, 8
N_CORES = 8
MG, EG = 2, 4                  # m-groups x e-groups of cores
MROWS = (B * T) // MG          # 8192 rows per core
EC = DIM // EG                 # 512 e-cols per core
NQ = 4                         # x quarters per core
NMC = (MROWS // NQ) // 512     # 4 m-chunks of 512 per quarter
NT = DIM // 128                # 16 contraction tiles
NTH = NT // 2                  # 8 d-tiles per half
NET = EC // 128                # 4 e-tiles per core
SCALE = 1.0 / math.sqrt(RANK)

BF16 = ml_dtypes.bfloat16
_CACHE = {}


def _build():
    import concourse.bacc as bacc
    import concourse.mybir as mybir
    import concourse.tile as tile

    f32 = mybir.dt.float32
    bf16 = mybir.dt.bfloat16

    nc = bacc.Bacc(
        "TRN2", target_bir_lowering=False, debug=False, num_devices=N_CORES
    )
    # x^T quarters: [q, p, t, mc, m] with d = t*128+p, row = q*2048 + mc*512 + m
    xh = nc.dram_tensor(
        "xh", [NQ, 128, NT, NMC, 512], bf16, kind="ExternalInput"
    ).ap()
    # factor slice: [et, dh, p, r, th, e] with d = dh*1024 + th*128 + p,
    # e_local = et*128+e.  One 2 MB dma per (et, dh), 16 KB per-partition
    # lines.
    fh = nc.dram_tensor(
        "fh", [NET, 2, 128, RANK, NTH, 128], bf16, kind="ExternalInput"
    ).ap()
    # bias_cols[p, et] = bias[eq*512 + et*128 + p]
    bias_cols = nc.dram_tensor("bias_cols", [128, NET], f32, kind="ExternalInput").ap()
    # transposed output: outT[e_local, m_local], bf16
    outT = nc.dram_tensor("outT", [EC, MROWS], bf16, kind="ExternalOutput").ap()

    with tile.TileContext(nc) as tc:
        with (
            tc.tile_pool(name="const", bufs=1) as const_pool,
            tc.tile_pool(name="xa", bufs=2) as xa_pool,
            tc.tile_pool(name="fr", bufs=3) as fr_pool,
            tc.tile_pool(name="acc", bufs=1) as acc_pool,
            tc.tile_pool(name="w", bufs=8) as w_pool,
            tc.tile_pool(name="osb", bufs=2) as o_pool,
            tc.tile_pool(name="ps", bufs=2, space="PSUM") as p_pool,
        ):
            scope = nc.named_scope
            bias_sb = const_pool.tile([128, NET], f32)
            # gpsimd ring: idle until the first out-DMA (~55us), so the
            # bias load never head-blocks the fh0 descriptors on sync
            nc.gpsimd.dma_start(bias_sb[:], bias_cols[:])

            for it in range(REPEAT):
                # wts[et*2 + dh] = W[d in half dh, e-tile et], [128, NTH, 128]
                wts = [None] * (NET * 2)
                fr_tiles = {}

                def issue_fh(et):
                    # both d-halves on the fh-dedicated rings (16 KB lines)
                    for dh in range(2):
                        fr = fr_pool.tile(
                            [128, RANK, NTH, 128], bf16, tag="fr",
                            name=f"fr{it}_{et}_{dh}",
                        )
                        eng = nc.sync if dh == 0 else nc.scalar
                        eng.dma_start(fr[:], fh[et, dh])
                        fr_tiles[(et, dh)] = fr

                def reduce_chains(et):
                    # serial DVE chain per d-half (~4.9us after the fh
                    # piece lands; GpSimd "parallel" halves measured
                    # slower — Pool adds have high per-op cost)
                    for dh in range(2):
                        fr = fr_tiles[(et, dh)]
                        with scope(f"red{it}_{et}_{dh}"):
                            a1 = acc_pool.tile(
                                [128, NTH, 128], bf16, tag="a1",
                                name=f"a1_{it}_{et}_{dh}",
                            )
                            nc.vector.tensor_add(a1[:], fr[:, 0], fr[:, 1])
                            for r in range(2, RANK - 1):
                                nc.vector.tensor_add(a1[:], a1[:], fr[:, r])
                            w = w_pool.tile(
                                [128, NTH, 128], bf16, tag="w",
                                name=f"w{it}_{et}_{dh}",
                            )
                            nc.vector.tensor_add(w[:], a1[:], fr[:, RANK - 1])
                            wts[et * 2 + dh] = w

                def issue_x(q, xa, pieces):
                    # pieces of 4 d-tiles (2 MB, 16 KB lines) alternating
                    # the two input rings; the d-loop consumes them in
                    # order. (Aggregate early DMA is ~300-330 GB/s per
                    # core — HBM pair-shared — so ring games don't help;
                    # this interleave empirically starts the PE earliest.)
                    for h in pieces:
                        eng = nc.sync if h % 2 == 0 else nc.scalar
                        eng.dma_start(
                            xa[:, 4 * h : 4 * h + 4], xh[q, :, 4 * h : 4 * h + 4]
                        )

                xa_tiles = []

                def new_xa(q):
                    t = xa_pool.tile(
                        [128, NT, NMC, 512], bf16, tag="xa", name=f"xa{it}_{q}"
                    )
                    xa_tiles.append(t)
                    return t

                # Ring order (per-ring FIFO): fh0, all of x0, fh1, fh2,
                # fh3, x1-3. The early phase is supply-bound either way;
                # loading x0 contiguously consolidates the unavoidable
                # supply stalls into ONE gap (before et1) instead of four,
                # so the PE p-state ramp (~1.5us per restart) is paid once.
                # Outs ride gpsimd.
                issue_fh(0)
                xa0 = new_xa(0)
                issue_x(0, xa0, [0, 1, 2, 3])
                issue_fh(1)
                issue_fh(2)
                issue_fh(3)
                for q in range(1, NQ):
                    issue_x(q, new_xa(q), [0, 1, 2, 3])
                reduce_chains(0)
                reduce_chains(1)

                # GEMM: out^T[e-tile, quarter]; 4 m-chunks share each
                # stationary W d-tile. et2/et3 reduce chains are issued
                # between the first GEMM groups so no engine queue holds
                # early evictions hostage behind late-gated chain adds.
                for q in range(NQ):
                    xa = xa_tiles[q]
                    for et in range(NET):
                        if q == 0 and et == 1:
                            reduce_chains(2)
                        if q == 0 and et == 2:
                            reduce_chains(3)
                        last = q == NQ - 1 and et == NET - 1
                        with scope(f"g{it}_{q}_{et}"):
                            ps = p_pool.tile([128, NMC, 512], f32, tag="ps")
                            if last:
                                # final group: two G=2 sub-groups so the
                                # first half's evictions + out-DMA overlap
                                # the second half's matmuls (shorter tail)
                                mc_waves = [[0, 1], [2, 3]]
                            else:
                                mc_waves = [list(range(NMC))]
                            for wave in mc_waves:
                                for d in range(NT):
                                    wt = wts[et * 2 + d // NTH]
                                    for mc in wave:
                                        nc.tensor.matmul(
                                            ps[:, mc, :],
                                            wt[:, d % NTH, :],
                                            xa[:, d, mc, :],
                                            start=(d == 0),
                                            stop=(d == NT - 1),
                                        )
                            osb = o_pool.tile([128, NMC * 512], bf16, tag="osb")
                            for mc in range(NMC):
                                # q0 evictions stay on Scalar: the Vector
                                # queue is still draining reduce chains and
                                # would hold the PSUM release hostage.
                                dst = osb[:, mc * 512 : (mc + 1) * 512]
                                src = ps[:, mc, :]
                                if q == 0 or mc % 2:
                                    nc.scalar.activation(
                                        dst,
                                        src,
                                        mybir.ActivationFunctionType.Identity,
                                        bias=bias_sb[:, et : et + 1],
                                        scale=SCALE,
                                    )
                                else:
                                    nc.vector.tensor_scalar(
                                        dst,
                                        src,
                                        SCALE,
                                        bias_sb[:, et : et + 1],
                                        mybir.AluOpType.mult,
                                        mybir.AluOpType.add,
                                    )
                            e0 = et * 128
                            m0 = q * NMC * 512
                            # two half-DMAs on the gpsimd ring (never behind
                            # WAR-gated x pieces): the last half only waits
                            # on the last two bank evictions, shortening the
                            # tail.
                            half = NMC * 512 // 2
                            nc.gpsimd.dma_start(
                                outT[e0 : e0 + 128, m0 : m0 + half], osb[:, :half]
                            )
                            nc.gpsimd.dma_start(
                                outT[e0 : e0 + 128, m0 + half : m0 + 2 * half],
                                osb[:, half:],
                            )

    nc.compile()
    return nc


def _get_nc():
    if "nc" not in _CACHE:
        _CACHE["nc"] = _build()
    return _CACHE["nc"]


def _shard(x, factors, bias):
    x_flat = np.ascontiguousarray(x, dtype=np.float32).reshape(B * T, DIM)
    factors = np.ascontiguousarray(factors, dtype=np.float32)
    bias = np.ascontiguousarray(bias, dtype=np.float32)
    in_maps = []
    for c in range(N_CORES):
        mh, eq = divmod(c, EG)
        xc = x_flat[mh * MROWS : (mh + 1) * MROWS, :]       # [m, d]
        # -> [q, p, t, mc, m512]
        xhc = np.ascontiguousarray(
            xc.reshape(NQ, NMC, 512, NT, 128).transpose(0, 4, 3, 1, 2).astype(BF16)
        )
        fc = factors[:, :, eq * EC : (eq + 1) * EC]          # [r, d, e]
        # -> [et, dh, p, r, th, e128]
        fhc = np.ascontiguousarray(
            fc.reshape(RANK, 2, NTH, 128, NET, 128)
            .transpose(4, 1, 3, 0, 2, 5)
            .astype(BF16)
        )
        bias_c = np.ascontiguousarray(
            bias[eq * EC : (eq + 1) * EC].reshape(NET, 128).T
        )
        in_maps.append({"xh": xhc, "fh": fhc, "bias_cols": bias_c})
    return in_maps


def _run(in_maps, trace=False, trace_cores=None):
    from concourse.bass_utils import run_bass_kernel_spmd

    nc = _get_nc()
    return run_bass_kernel_spmd(
        nc, in_maps, list(range(N_CORES)), trace=trace, trace_cores=trace_cores
    )


def _assemble(res):
    out = np.empty((B * T, DIM), dtype=np.float32)
    for c in range(N_CORES):
        mh, eq = divmod(c, EG)
        out[mh * MROWS : (mh + 1) * MROWS, eq * EC : (eq + 1) * EC] = (
            res.results[c]["outT"].T.astype(np.float32)
        )
    return out.reshape(B, T, DIM)


def kernel(x, factors, bias):
    res = _run(_shard(x, factors, bias), trace=False)
    return _assemble(res)
